# revision 1
# baseline (speedup 1.0000x reference)
"""Multi-head causal attention (b=4, l=2048, d=1024, 16 heads x 64) on 8 trn2 cores.

Sharding: core c handles batch (c // 2) and head-group (c % 2) of 8 heads.
Each core computes a partial output x[b] @ W (its 8 heads' contribution);
the host sums the two partials per batch.

Device layouts (per core):
  xT      [1024, 2048]   x[b] transposed on host (d on partitions)
  wq/wk   [1024, 512]    head-group column slices (natural layout, lhsT)
  wv      [1024, 512]
  wo      [512, 1024]    head-group row slice (rhs)
  qT/kT   [512, 2048]    c on partitions (4 sbuf tensors of 128)
  v_pad   16 x [128, 8, 65]  v natural (l on partitions), per head 64 cols + ones col
  S^T     [128 m, 512 q] tiles -> exp -> P^T; PV: O'^T = [V|1]^T P^T gives sums row
  softmax uses no max-subtraction (scores are O(1)); the fully-masked q=0
  column is fixed up with a uniform mean-of-V matmul.

All matmul inputs are float32r (~tf32 precision, 4x faster than fp32 on PE).
"""

import os
import sys

sys.path.insert(0, "/opt/trn_rl_repo")

import numpy as np

import concourse.bacc as bacc
import concourse.mybir as mybir
import concourse.tile as tile
from concourse.bass_utils import run_bass_kernel_spmd

F32 = mybir.dt.float32
F32R = mybir.dt.float32r
AF = mybir.ActivationFunctionType
ALU = mybir.AluOpType

B, L, D = 4, 2048, 1024
N_HEAD, KEY_DIM = 16, 64
HG = 8               # heads per core (head-group)
C = HG * KEY_DIM     # 512 per-core qkv width
SCALE = 1.0 / 8.0    # 1/sqrt(KEY_DIM)
NLC = 16             # l chunks of 128
NJ = 4               # q chunks of 512
ND = 8               # d chunks of 128
NCC = 4              # c chunks of 128

_CACHED = {}


def build_nc():
    nc = bacc.Bacc("TRN2", target_bir_lowering=False, debug=False)

    xT = nc.dram_tensor("xT", [D, L], F32R, kind="ExternalInput")
    wq = nc.dram_tensor("wq", [D, C], F32R, kind="ExternalInput")
    wk = nc.dram_tensor("wk", [D, C], F32R, kind="ExternalInput")
    wv = nc.dram_tensor("wv", [D, C], F32R, kind="ExternalInput")
    wo = nc.dram_tensor("wo", [C, D], F32R, kind="ExternalInput")
    out = nc.dram_tensor("out", [L, D], F32, kind="ExternalOutput")

    with tile.TileContext(nc) as tc:
        # ---- persistent pools (live across all phases) ----
        with tc.tile_pool(name="persist", bufs=1) as persist, \
             tc.tile_pool(name="const", bufs=1) as constp:

            qT = [persist.tile([128, L], F32R, name=f"qT{t}") for t in range(NCC)]
            kT = [persist.tile([128, L], F32R, name=f"kT{t}") for t in range(NCC)]
            vp = [persist.tile([128, HG, KEY_DIM + 1], F32R, name=f"vp{i}")
                  for i in range(NLC)]
            masks = [constp.tile([128, 512], F32, name=f"mask{r}") for r in range(4)]

            # constants
            for r in range(4):
                nc.gpsimd.memset(masks[r][:], 1.0)
                # keep where f - p - 128*r > 0 (i.e. q > m), else 0
                nc.gpsimd.affine_select(
                    out=masks[r][:], in_=masks[r][:],
                    compare_op=ALU.is_gt, fill=0.0,
                    base=-(128 * r), channel_multiplier=-1, pattern=[[1, 512]],
                )
            for i in range(NLC):
                # whole-tile memset (strided column memset fails ISA check);
                # phase-2 copies overwrite cols 0..63, col 64 stays 1.0
                nc.vector.memset(vp[i][:].bitcast(F32), 1.0)

            # ---- phase 1+2: stream xT, project q/k/v ----
            with tc.tile_pool(name="wqkv", bufs=1) as wpool, \
                 tc.tile_pool(name="xt", bufs=8) as xtp, \
                 tc.tile_pool(name="psA", bufs=3, space="PSUM") as psA:
                wq_sb = [wpool.tile([128, C], F32R, name=f"wq{d}") for d in range(ND)]
                wk_sb = [wpool.tile([128, C], F32R, name=f"wk{d}") for d in range(ND)]
                wv_sb = [wpool.tile([128, C], F32R, name=f"wv{d}") for d in range(ND)]
                for d in range(ND):
                    nc.sync.dma_start(wq_sb[d][:], wq[128 * d:128 * (d + 1), :])
                    nc.sync.dma_start(wk_sb[d][:], wk[128 * d:128 * (d + 1), :])
                    nc.sync.dma_start(wv_sb[d][:], wv[128 * d:128 * (d + 1), :])

                for lc in range(NJ):  # 4 l-chunks of 512
                    ls = slice(512 * lc, 512 * (lc + 1))
                    xts = []
                    for d in range(ND):
                        t = xtp.tile([128, 512], F32R, name=f"xt{lc}_{d}", tag="xt")
                        nc.sync.dma_start(t[:], xT[128 * d:128 * (d + 1), ls])
                        xts.append(t)
                    # qT / kT chunks: out [128 c, 512 l]
                    for w_sb, dst in ((wq_sb, qT), (wk_sb, kT)):
                        for cc in range(NCC):
                            ps = psA.tile([128, 512], F32, name=f"pqk{lc}{cc}", tag="psA")
                            for d in range(ND):
                                nc.tensor.matmul(
                                    ps[:], w_sb[d][:, 128 * cc:128 * (cc + 1)],
                                    xts[d][:], start=(d == 0), stop=(d == ND - 1))
                            nc.scalar.copy(dst[cc][:, ls], ps[:])
                    # v natural: out [128 l, 512 c] -> strided into vp
                    for lcc in range(4):
                        i = 4 * lc + lcc
                        ps = psA.tile([128, 512], F32, name=f"pv{i}", tag="psA")
                        for d in range(ND):
                            nc.tensor.matmul(
                                ps[:], xts[d][:, 128 * lcc:128 * (lcc + 1)],
                                wv_sb[d][:], start=(d == 0), stop=(d == ND - 1))
                        nc.scalar.copy(
                            vp[i][:, :, 0:KEY_DIM],
                            ps[:].rearrange("p (h c) -> p h c", h=HG))

            # ---- phase 3+4 (j-major): attention + output projection ----
            with tc.tile_pool(name="of", bufs=1) as ofp, \
                 tc.tile_pool(name="wo", bufs=1) as wop, \
                 tc.tile_pool(name="pp", bufs=10) as pp, \
                 tc.tile_pool(name="ep", bufs=3) as ep, \
                 tc.tile_pool(name="bcp", bufs=2) as bcp, \
                 tc.tile_pool(name="ovp", bufs=4) as ovp, \
                 tc.tile_pool(name="osb", bufs=3) as osb, \
                 tc.tile_pool(name="rp", bufs=2) as rp, \
                 tc.tile_pool(name="psS", bufs=4, space="PSUM") as psS, \
                 tc.tile_pool(name="psO", bufs=2, space="PSUM") as psO, \
                 tc.tile_pool(name="psF", bufs=2, space="PSUM") as psF:

                OF = [ofp.tile([128, L], F32R, name=f"of{t}") for t in range(NCC)]
                wo_sb = [wop.tile([128, D], F32R, name=f"wo{t}") for t in range(NCC)]
                for t in range(NCC):
                    nc.sync.dma_start(wo_sb[t][:], wo[128 * t:128 * (t + 1), :])

                for j in range(NJ):
                    js = slice(512 * j, 512 * (j + 1))
                    n_i = 4 * j + 4
                    for hp in range(4):  # head pairs share kT/qT tensor hp
                        # even head on partitions 0:64 (PE tile T0), odd head
                        # on 64:128 (T8): alternating their 64-row S matmuls
                        # runs them concurrently on the two array halves.
                        o_ps = {}
                        for z in range(2):
                            o_ps[z] = psO.tile([65, 512], F32,
                                               name=f"o{j}{hp}{z}", tag="psO")
                        p_tiles = {0: [], 1: []}
                        for ib in range(0, n_i, 4):
                            ie = min(ib + 4, n_i)
                            for i in range(ib, ie):
                                for z in range(2):
                                    rows = slice(64 * z, 64 * z + 64)
                                    s_ps = psS.tile([128, 512], F32,
                                                    name=f"s{j}{hp}{i}{z}",
                                                    tag="psS")
                                    nc.tensor.matmul(
                                        s_ps[:],
                                        kT[hp][rows, 128 * i:128 * (i + 1)],
                                        qT[hp][rows, js], start=True, stop=True)
                                    p_sb = pp.tile([128, 512], F32R,
                                                   name=f"p{j}{hp}{i}{z}",
                                                   tag="pp")
                                    if i >= 4 * j:  # mixed tile: mask post-exp
                                        e_sb = ep.tile([128, 512], F32,
                                                       name=f"e{j}{hp}{i}{z}",
                                                       tag="ep")
                                        nc.scalar.activation(e_sb[:], s_ps[:],
                                                             AF.Exp, scale=SCALE)
                                        nc.vector.tensor_tensor(
                                            p_sb[:], e_sb[:],
                                            masks[i - 4 * j][:], op=ALU.mult)
                                    else:
                                        nc.scalar.activation(p_sb[:], s_ps[:],
                                                             AF.Exp, scale=SCALE)
                                    p_tiles[z].append(p_sb)
                            for i in range(ib, ie):
                                for z in range(2):
                                    nc.tensor.matmul(
                                        o_ps[z][:], vp[i][:, 2 * hp + z, :],
                                        p_tiles[z][i][:],
                                        start=(i == 0), stop=(i == n_i - 1))
                        # evacuate O' psum quickly (frees the bank), then
                        # normalize rows 0..63 by sums row 64 from SBUF
                        for z in range(2):
                            rows = slice(64 * z, 64 * z + 64)
                            ov_sb = ovp.tile([65, 512], F32,
                                             name=f"ov{j}{hp}{z}", tag="ovp")
                            nc.scalar.copy(ov_sb[:], o_ps[z][:])
                            r_sb = rp.tile([1, 512], F32, name=f"r{j}{hp}{z}",
                                           tag="rp")
                            nc.vector.reciprocal(r_sb[:], ov_sb[64:65, :])
                            bc_sb = bcp.tile([64, 512], F32,
                                             name=f"bc{j}{hp}{z}", tag="bcp")
                            nc.gpsimd.partition_broadcast(bc_sb[:], r_sb[:])
                            nc.vector.tensor_tensor(
                                OF[hp][rows, js], ov_sb[0:64, :], bc_sb[:],
                                op=ALU.mult)
                    # ---- phase 4 for the q-chunks completed by this j ----
                    for qc in range(4 * j, 4 * j + 4):
                        qs = slice(128 * qc, 128 * (qc + 1))
                        for n in range(2):
                            ns = slice(512 * n, 512 * (n + 1))
                            f_ps = psF.tile([128, 512], F32,
                                            name=f"f{qc}{n}", tag="psF")
                            for t in range(NCC):
                                nc.tensor.matmul(
                                    f_ps[:], OF[t][:, qs], wo_sb[t][:, ns],
                                    start=(t == 0), stop=(t == NCC - 1))
                            o_sb = osb.tile([128, 512], F32,
                                            name=f"ob{qc}{n}", tag="osb")
                            nc.scalar.copy(o_sb[:], f_ps[:])
                            nc.sync.dma_start(out[qs, ns], o_sb[:])

    nc.finalize()
    return nc


def _get_nc():
    if "nc" not in _CACHED:
        _CACHED["nc"] = build_nc()
    return _CACHED["nc"]


def kernel(x, W_q, W_k, W_v, W_out, trace=False, trace_kwargs=None):
    x = np.asarray(x, dtype=np.float32)
    W_q = np.asarray(W_q, dtype=np.float32)
    W_k = np.asarray(W_k, dtype=np.float32)
    W_v = np.asarray(W_v, dtype=np.float32)
    W_out = np.asarray(W_out, dtype=np.float32)

    nc = _get_nc()
    in_maps = []
    for core in range(8):
        b, g = core // 2, core % 2
        cs = slice(C * g, C * (g + 1))
        in_maps.append({
            "xT": np.ascontiguousarray(x[b].T),
            "wq": np.ascontiguousarray(W_q[:, cs]),
            "wk": np.ascontiguousarray(W_k[:, cs]),
            "wv": np.ascontiguousarray(W_v[:, cs]),
            "wo": np.ascontiguousarray(W_out[cs, :]),
        })
    res = run_bass_kernel_spmd(nc, in_maps, core_ids=list(range(8)),
                               trace=trace, **(trace_kwargs or {}))
    out = np.empty((B, L, D), dtype=np.float32)
    for b in range(B):
        out[b] = res.results[2 * b]["out"] + res.results[2 * b + 1]["out"]
        # q=0 is fully masked -> reference softmax gives uniform attention over
        # all of V; the device leaves NaN/0 in that row, patch it here.
        out[b, 0, :] = (x[b].mean(axis=0) @ W_v) @ W_out
    if trace:
        return out, res
    return out



# revision 12
# speedup vs baseline: 1.8503x; 1.8503x over previous
"""Multi-head causal attention (b=4, l=2048, d=1024, 16 heads x 64) on 8 trn2 cores.

Sharding: core c handles batch (c // 2) and head-group (c % 2) of 8 heads.
Each core computes a partial output x[b] @ W (its 8 heads' contribution);
the host sums the two partials per batch.

v2 design (vs fp32r baseline):
  - full bf16 datapath (PSUM stays fp32): enables PE fast-weight-load and
    halves DMA/SBUF traffic; accuracy is well within the 2e-2 gate.
  - single fused instruction stream: projections for l-chunk lc=j+1 and the
    output projection for q-block j-1 are interleaved (via a filler FIFO)
    into attention block j's S/PV matmuls, keeping the PE dense so it holds
    the fast HAM p-state and hides the exp (ACT) latency.
  - S tiles for the two heads of a pair land in one 2-bank PSUM tile and are
    exp'd with ONE activation instruction (halves ACT instruction overhead).
  - causal masking via gpsimd.affine_select on the diagonal 128x128 block of
    P (post-exp, fill=0); off-diagonal-block columns are skipped entirely
    (S / exp / PV all shortened to the unmasked column span).
  - softmax denominators via the [V|1] ones-column trick; normalization uses
    reciprocal_approx_fast (the exact DVE reciprocal is ~3.3us per call).

Device layouts (per core):
  xT      [1024, 2048]  x[b]^T (d on partitions), 32 sbuf tiles [128,512]
  wq/wk/wv [1024, 512]  head-group column slices; wo [512, 1024] row slice
  qT/kT   4 x [128, 2048]  c on partitions (head-pair per tile)
  vp      16 x [128, 8, 65]  v natural (l on partitions), per head 64 + ones
  OF      4 x [128, 2048]  normalized attention output (pre-out-proj)
  S^T     [128 m, 2x512 q] psum pairs -> exp -> P^T bf16; PV: [V|1]^T P^T
  q=0 is fully masked; its softmax sum is 0 and the garbage column is fixed
  up on the host (row 0 of the output only).
"""

import sys

sys.path.insert(0, "/opt/trn_rl_repo")

import ml_dtypes
import numpy as np

import concourse.bacc as bacc
import concourse.mybir as mybir
import concourse.tile as tile
from concourse.bass_utils import run_bass_kernel_spmd

F32 = mybir.dt.float32
BF16 = mybir.dt.bfloat16
AF = mybir.ActivationFunctionType
ALU = mybir.AluOpType

B, L, D = 4, 2048, 1024
N_HEAD, KEY_DIM = 16, 64
HG = 8               # heads per core (head-group)
C = HG * KEY_DIM     # 512 per-core qkv width
SCALE = 1.0 / 8.0    # 1/sqrt(KEY_DIM)
ND = 8               # d chunks of 128
NJ = 4               # q blocks of 512
NCC = 4              # c chunks of 128 (= head pairs)

_CACHED = {}


def build_nc():
    nc = bacc.Bacc("TRN2", target_bir_lowering=False, debug=False)

    xT = nc.dram_tensor("xT", [D, L], BF16, kind="ExternalInput")
    wq = nc.dram_tensor("wq", [D, C], BF16, kind="ExternalInput")
    wk = nc.dram_tensor("wk", [D, C], BF16, kind="ExternalInput")
    wv = nc.dram_tensor("wv", [D, C], BF16, kind="ExternalInput")
    wo = nc.dram_tensor("wo", [C, D], BF16, kind="ExternalInput")
    out = nc.dram_tensor("out", [L, D], F32, kind="ExternalOutput")

    with tile.TileContext(nc) as tc:
        with tc.tile_pool(name="wp", bufs=1) as wp, \
             tc.tile_pool(name="xp", bufs=1) as xp, \
             tc.tile_pool(name="qkv", bufs=1) as qkv, \
             tc.tile_pool(name="ofp", bufs=1) as ofp, \
             tc.tile_pool(name="pp", bufs=16) as pp, \
             tc.tile_pool(name="ovp", bufs=9) as ovp, \
             tc.tile_pool(name="smp", bufs=1) as smp, \
             tc.tile_pool(name="bcp", bufs=2) as bcp, \
             tc.tile_pool(name="osb", bufs=2) as osb, \
             tc.tile_pool(name="psS", bufs=2, space="PSUM") as psS, \
             tc.tile_pool(name="psX", bufs=2, space="PSUM") as psX, \
             tc.tile_pool(name="psO", bufs=2, space="PSUM") as psO:

            wq_sb = [wp.tile([128, C], BF16, name=f"wq{d}") for d in range(ND)]
            wk_sb = [wp.tile([128, C], BF16, name=f"wk{d}") for d in range(ND)]
            wv_sb = [wp.tile([128, C], BF16, name=f"wv{d}") for d in range(ND)]
            wo_sb = [wp.tile([128, D], BF16, name=f"wo{t}") for t in range(NCC)]
            xt = [[xp.tile([128, 512], BF16, name=f"xt{lc}_{d}")
                   for d in range(ND)] for lc in range(NJ)]
            qT = [qkv.tile([128, L], BF16, name=f"qT{t}") for t in range(NCC)]
            kT = [qkv.tile([128, L], BF16, name=f"kT{t}") for t in range(NCC)]
            vp = [qkv.tile([128, HG, KEY_DIM + 1], BF16, name=f"vp{i}")
                  for i in range(16)]
            OF = [ofp.tile([128, L], BF16, name=f"of{t}") for t in range(NCC)]

            # input DMA: qkv weights + x l-chunk 0 first, then the rest
            for d in range(ND):
                nc.sync.dma_start(wq_sb[d][:], wq[128 * d:128 * (d + 1), :])
                nc.sync.dma_start(wk_sb[d][:], wk[128 * d:128 * (d + 1), :])
                nc.sync.dma_start(wv_sb[d][:], wv[128 * d:128 * (d + 1), :])
            for lc in range(NJ):
                ls = slice(512 * lc, 512 * (lc + 1))
                for d in range(ND):
                    nc.sync.dma_start(xt[lc][d][:], xT[128 * d:128 * (d + 1), ls])
            for t in range(NCC):
                nc.sync.dma_start(wo_sb[t][:], wo[128 * t:128 * (t + 1), :])

            # ones column for the softmax-denominator trick (copies below
            # overwrite cols 0..63 of each head; col 64 stays 1.0)
            for i in range(16):
                nc.vector.memset(vp[i][:], 1.0)

            # ---------- filler units (issued between attention matmuls) ----
            def proj_chain(lc, c):
                """Chain c of 12 for l-chunk lc: c in 0..7 -> q/k interleaved
                (wq cc, wk cc, ...), c in 8..11 -> v l-subchunk."""
                ls = slice(512 * lc, 512 * (lc + 1))
                ps = psX.tile([128, 512], F32, name=f"pj{lc}_{c}", tag="psX")
                if c < 8:
                    w_sb, dst = ((wq_sb, qT) if c % 2 == 0 else (wk_sb, kT))
                    cc = c // 2
                    for d in range(ND):
                        nc.tensor.matmul(
                            ps[:], w_sb[d][:, 128 * cc:128 * (cc + 1)],
                            xt[lc][d][:], start=(d == 0), stop=(d == ND - 1))
                    cp = nc.scalar.copy if lc == 0 else nc.vector.tensor_copy
                    cp(dst[cc][:, ls], ps[:])
                else:
                    lcc = c - 8
                    i = 4 * lc + lcc
                    for d in range(ND):
                        nc.tensor.matmul(
                            ps[:], xt[lc][d][:, 128 * lcc:128 * (lcc + 1)],
                            wv_sb[d][:], start=(d == 0), stop=(d == ND - 1))
                    cp = nc.scalar.copy if lc == 0 else nc.vector.tensor_copy
                    cp(vp[i][:, :, 0:KEY_DIM],
                       ps[:].rearrange("p (h c) -> p h c", h=HG))

            def ph4_unit(qc, n):
                """Output projection for q-chunk qc (128 rows), d-half n."""
                qs = slice(128 * qc, 128 * (qc + 1))
                ns = slice(512 * n, 512 * (n + 1))
                f_ps = psX.tile([128, 512], F32, name=f"f{qc}_{n}", tag="psX")
                for t in range(NCC):
                    nc.tensor.matmul(f_ps[:], OF[t][:, qs], wo_sb[t][:, ns],
                                     start=(t == 0), stop=(t == NCC - 1))
                o_sb = osb.tile([128, 512], F32, name=f"ob{qc}_{n}", tag="osb")
                nc.vector.tensor_copy(o_sb[:], f_ps[:])
                nc.sync.dma_start(out[qs, ns], o_sb[:])

            # ---------- phase 0: projections for l-chunk 0 ----------------
            for c in range(12):
                proj_chain(0, c)

            # ---------- fused attention loop ------------------------------
            for j in range(NJ):
                js = slice(512 * j, 512 * (j + 1))
                n_i = 4 * j + 4
                fifo = []
                if j + 1 < NJ:
                    fifo += [("proj", j + 1, c) for c in range(12)]
                if j > 0:
                    fifo += [("ph4", qc, n)
                             for qc in range(4 * (j - 1), 4 * j)
                             for n in range(2)]

                def pop_filler(idx):
                    while fifo:
                        kind = fifo[0][0]
                        if kind == "ph4" and idx is not None and idx < 7:
                            return  # OF of prev block may not be ready yet
                        u = fifo.pop(0)
                        if u[0] == "proj":
                            proj_chain(u[1], u[2])
                        else:
                            ph4_unit(u[1], u[2])
                        return

                # sums rows live at quadrant-aligned partitions (0/32/64/96):
                # engine partition bases must be multiples of 32
                sums = [smp.tile([128, 512], F32, name=f"sums{j}{t}",
                                 tag=f"sums{t}") for t in range(2)]
                for t in range(2):
                    nc.gpsimd.memset(sums[t][:], 1.0)
                ovs = {}

                for hp in range(NCC):
                    p_tiles = []
                    # S + exp (+ causal select) for all key chunks
                    for i in range(n_i):
                        r = i - 4 * j  # >=0 on the diagonal blocks
                        off = 128 * r if r > 0 else 0
                        s_ps = psS.tile([128, 1024], F32,
                                        name=f"s{j}{hp}{i}", tag="psS")
                        for z in range(2):
                            rows = slice(64 * z, 64 * z + 64)
                            nc.tensor.matmul(
                                s_ps[:, 512 * z + off:512 * (z + 1)],
                                kT[hp][rows, 128 * i:128 * (i + 1)],
                                qT[hp][rows, 512 * j + off:512 * (j + 1)],
                                start=True, stop=True)
                        p_sb = pp.tile([128, 1024], BF16,
                                       name=f"p{j}{hp}{i}", tag="pp")
                        if off:
                            s3 = s_ps[:].rearrange("p (z w) -> p z w", z=2)
                            p3 = p_sb[:].rearrange("p (z w) -> p z w", z=2)
                            nc.scalar.activation(p3[:, :, off:512],
                                                 s3[:, :, off:512],
                                                 AF.Exp, scale=SCALE)
                        else:
                            nc.scalar.activation(p_sb[:], s_ps[:],
                                                 AF.Exp, scale=SCALE)
                        if r >= 0:
                            p3 = p_sb[:].rearrange("p (z w) -> p z w", z=2)
                            for z in range(2):
                                nc.gpsimd.affine_select(
                                    out=p3[:, z, off:off + 128],
                                    in_=p3[:, z, off:off + 128],
                                    compare_op=ALU.is_gt, fill=0.0,
                                    base=0, channel_multiplier=-1,
                                    pattern=[[1, 128]])
                        p_tiles.append((p_sb, off))
                        if i % 2 == 1:
                            pop_filler(i)
                    # PV: accumulate [V|1]^T P^T over all key chunks
                    o_ps = [psO.tile([65, 512], F32, name=f"o{j}{hp}{z}",
                                     tag="psO") for z in range(2)]
                    for i in range(n_i):
                        p_sb, off = p_tiles[i]
                        p3 = p_sb[:].rearrange("p (z w) -> p z w", z=2)
                        for z in range(2):
                            nc.tensor.matmul(
                                o_ps[z][:, off:512], vp[i][:, 2 * hp + z, :],
                                p3[:, z, off:512],
                                start=(i == 0), stop=(i == n_i - 1))
                    pop_filler(None)
                    # evacuate O' and collect the sums rows (row 64) into one
                    # [8, 512] tile; one exact reciprocal per j covers all 8
                    # (reciprocal cost scales with per-partition elements)
                    for z in range(2):
                        flat = 2 * hp + z
                        ov = ovp.tile([65, 512], F32, name=f"ov{j}{hp}{z}",
                                      tag="ovp")
                        nc.vector.tensor_copy(ov[:], o_ps[z][:])
                        row = 32 * (flat % 4)
                        nc.vector.tensor_copy(
                            sums[flat // 4][row:row + 1, :], ov[64:65, :])
                        ovs[(hp, z)] = ov
                rec = [smp.tile([128, 512], F32, name=f"rec{j}{t}",
                                tag=f"rec{t}") for t in range(2)]
                for t in range(2):
                    nc.vector.reciprocal(rec[t][:], sums[t][:])
                for hp in range(NCC):
                    for z in range(2):
                        flat = 2 * hp + z
                        row = 32 * (flat % 4)
                        # partition_broadcast is only proven for base-0
                        # inputs: stage the quadrant row down to partition 0
                        r1 = bcp.tile([1, 512], F32, name=f"r1{j}{hp}{z}",
                                      tag="r1")
                        nc.vector.tensor_copy(
                            r1[:], rec[flat // 4][row:row + 1, :])
                        bc = bcp.tile([64, 512], F32, name=f"bc{j}{hp}{z}",
                                      tag="bcp")
                        nc.gpsimd.partition_broadcast(bc[:], r1[:])
                        nc.vector.tensor_tensor(
                            OF[hp][64 * z:64 * z + 64, js],
                            ovs[(hp, z)][0:64, :], bc[:], op=ALU.mult)
                while fifo:
                    pop_filler(None)

            # output projection for the last q block
            for qc in range(12, 16):
                for n in range(2):
                    ph4_unit(qc, n)

    nc.finalize()
    return nc


def _get_nc():
    if "nc" not in _CACHED:
        _CACHED["nc"] = build_nc()
    return _CACHED["nc"]


def kernel(x, W_q, W_k, W_v, W_out, trace=False, trace_kwargs=None):
    x = np.asarray(x, dtype=np.float32)
    W_q = np.asarray(W_q, dtype=np.float32)
    W_k = np.asarray(W_k, dtype=np.float32)
    W_v = np.asarray(W_v, dtype=np.float32)
    W_out = np.asarray(W_out, dtype=np.float32)
    bf = ml_dtypes.bfloat16

    nc = _get_nc()
    in_maps = []
    for core in range(8):
        b, g = core // 2, core % 2
        cs = slice(C * g, C * (g + 1))
        in_maps.append({
            "xT": np.ascontiguousarray(x[b].T.astype(bf)),
            "wq": np.ascontiguousarray(W_q[:, cs].astype(bf)),
            "wk": np.ascontiguousarray(W_k[:, cs].astype(bf)),
            "wv": np.ascontiguousarray(W_v[:, cs].astype(bf)),
            "wo": np.ascontiguousarray(W_out[cs, :].astype(bf)),
        })
    res = run_bass_kernel_spmd(nc, in_maps, core_ids=list(range(8)),
                               trace=trace, **(trace_kwargs or {}))
    out = np.empty((B, L, D), dtype=np.float32)
    for b in range(B):
        out[b] = res.results[2 * b]["out"] + res.results[2 * b + 1]["out"]
        # q=0 is fully masked -> reference softmax gives uniform attention over
        # all of V; the device leaves garbage in that row, patch it here.
        out[b, 0, :] = (x[b].mean(axis=0) @ W_v) @ W_out
    if trace:
        return out, res
    return out
